# revision 21
# baseline (speedup 1.0000x reference)
"""GIN message-passing kernel for Trainium2, 8-core SPMD.

Strategy (graph/data parallel, edges partitioned by destination):
 - Core m owns destination nodes [12500*m, 12500*(m+1)).
 - Edges sorted by (dst supertile, src chunk, dst tile) and padded to
   128-edge blocks with block counts equalized across cores so all 8
   cores run one identical program (SPMD); per-core variation is data.
 - Per layer: dma_gather fp16 source rows from the full node table
   (chunks of <=32768 rows for int16 indices), segment-sum via PSUM-
   accumulated matmuls  agg[feat, dst] += G_block.T @ onehot(dst_local),
   with a [128, 128] fp16 one-hot built on DVE in the 2x perf mode via
   tensor_scalar is_equal against an iota row. Gather tables are fp16
   (rel err ~2e-3); the residual/MLP path stays fp32.
 - MLP: feature-major matmuls with the 128x128 weights stationary; BN
   (eval) folded into per-partition scale/bias of a Relu activation.
 - Node-major h for the next layer's gather written via PE transpose;
   shards exchanged with an 8-core AllGather collective.
 - Pooling: per-graph-slot gathers (pad indices duplicate a real row of
   the same slot, neutral under max), PE transpose, reduce_max. The
   tiny [64,128] @ [128,4] classifier + log_softmax run on host.
"""

import os
import sys

if "/opt/trn_rl_repo" not in sys.path:
    sys.path.append("/opt/trn_rl_repo")

KM_DEBUG = os.environ.get("KM_DEBUG", "0") == "1"

from contextlib import ExitStack

import numpy as np

from concourse import bacc, mybir, tile
from concourse.bass_utils import run_bass_kernel_spmd
from concourse.masks import make_identity

F32 = mybir.dt.float32
F16 = mybir.dt.float16
I16 = mybir.dt.int16

N = 100_000
E = 3_200_000
D = 128
L = 3
K = 2
G = 64
C = 4
BN_EPS = 1e-5

NCORES = 8
SHARD = N // NCORES  # 12500
NTILES = 100  # 128-node dst tiles per core (12800 padded shard)
NSUP = 25  # supertiles of 512 dst nodes
NCHUNKS = 4  # gather chunks == AllGather quarter regions (max 28672 rows)
PAD_DST = 200.0  # local-dst sentinel for padded edges -> all-zero one-hot
NQUEUES = 4
GMAX = 36  # max 128-edge blocks per dma_gather call

# Shard quarters (supertile-aligned) for split AllGathers that overlap the
# tail of each layer's compute. The node table is stored PERMUTED:
# row(g) = QGOFF[q] + QSIZE[q]*m + (loc - QSTART[q]), where m = g//SHARD,
# loc = g%SHARD, q = quarter of loc. Each quarter's AllGather output is the
# rank-concatenation of that quarter across cores.
QSUP_LAST = [6, 12, 18, 24]  # last supertile of each quarter
QSTART = np.array([0, 3584, 6656, 9728])
QSIZE = np.array([3584, 3072, 3072, 2772])
QGOFF = np.array([0, 28672, 53248, 77824])


def _perm_rows(g):
    g = np.asarray(g, dtype=np.int64)
    m = g // SHARD
    loc = g % SHARD
    q = np.searchsorted(QSTART, loc, side="right") - 1
    return QGOFF[q] + QSIZE[q] * m + (loc - QSTART[q])

_compiled = {}


def _build_edge_layout(edge_index):
    """Per (supertile s, chunk c) call layout:

      [tr0 full blocks | tr1 | tr2 | tr3 | merged rest blocks]

    Full blocks are tr-pure (128-wide one-hot, dst value = loc%128, PAD 200).
    Each core contributes exactly 128*nbfull_tr = 128*floor(min_core cnt/128)
    edges per tr, so full blocks carry no padding at all. The remaining edges
    of all four trs form a contiguous per-core "rest" run consumed by merged
    blocks with a 512-wide one-hot (dst value = loc%512, PAD 600). The call's
    num_idxs is trimmed to 128*sum(nbfull) + max_core(rest), so the only
    transferred padding is the cross-core rest imbalance; slots beyond a
    core's rest (and beyond num_idxs) read stale SBUF/pad rows that the PAD
    one-hot value zeroes out.
    """
    src = np.asarray(edge_index[0], dtype=np.int64)
    dst = np.asarray(edge_index[1], dtype=np.int64)

    core = dst // SHARD
    dloc = dst - core * SHARD
    t = dloc // 128  # local dst tile, 0..97
    s = t // 4  # supertile
    trel = t - s * 4
    prow = _perm_rows(src)
    c = np.searchsorted(QGOFF, prow, side="right") - 1  # chunk == quarter

    key = ((core * NSUP + s) * NCHUNKS + c) * 4 + trel
    order = np.argsort(key, kind="stable")
    ks = key[order]

    ngroups = NCORES * NSUP * NCHUNKS * 4
    cnt = np.bincount(ks, minlength=ngroups).reshape(NCORES, NSUP, NCHUNKS, 4)

    # full blocks per (s, c, tr): floor of the min across cores
    nbfull = cnt.min(axis=0) // 128  # [NSUP, NCHUNKS, 4]
    rest = cnt - 128 * nbfull[None]  # [NCORES, NSUP, NCHUNKS, 4]
    rest_sc = rest.sum(axis=3)  # [NCORES, NSUP, NCHUNKS]
    maxrest = rest_sc.max(axis=0)  # [NSUP, NCHUNKS]
    nmerge = -(-maxrest // 128)  # ceil
    # every 128-col region of each supertile must be written by some matmul
    # (full blocks write their tr; merged blocks write all 512 cols)
    for si in range(NSUP):
        assert (nbfull[si].sum(axis=0) >= 1).all() or nmerge[si].sum() >= 1
    nb_sc = nbfull.sum(axis=2) + nmerge  # blocks per (s, c) call
    numidx = 128 * nbfull.sum(axis=2) + maxrest  # transferred descs per call

    nblk = int(nb_sc.sum())
    e_pad = nblk * 128

    # slot offsets: call-major (s, c); within call: tr fulls then rest
    call_off = np.zeros((NSUP, NCHUNKS), dtype=np.int64)
    call_off.reshape(-1)[1:] = np.cumsum((nb_sc * 128).reshape(-1))[:-1]
    tr_off = np.zeros((NSUP, NCHUNKS, 4), dtype=np.int64)
    tr_off[:, :, 1:] = np.cumsum(128 * nbfull, axis=2)[:, :, :-1]
    rest_off = call_off + 128 * nbfull.sum(axis=2)

    # per-edge slot: edges sorted by key; within (core,s,c,tr) group, first
    # 128*nbfull go to that tr's full slots, remainder appended to the
    # core's rest run (tr-ordered since groups are processed in tr order).
    grp_all = np.bincount(ks, minlength=ngroups)
    gstart = np.zeros(ngroups, dtype=np.int64)
    gstart[1:] = np.cumsum(grp_all)[:-1]
    within = np.arange(len(ks), dtype=np.int64) - gstart[ks]

    e_core = ks // (NSUP * NCHUNKS * 4)
    rem = ks % (NSUP * NCHUNKS * 4)
    e_s = rem // (NCHUNKS * 4)
    e_c = (rem // 4) % NCHUNKS
    e_tr = rem % 4

    nfull_e = 128 * nbfull[e_s, e_c, e_tr]
    is_full = within < nfull_e
    # rest position: edges of earlier trs in the same (core,s,c) that
    # overflowed, plus own overflow index
    rest_cum = np.cumsum(rest, axis=3) - rest  # rest before this tr
    rest_pos = rest_cum[e_core, e_s, e_c, e_tr] + (within - nfull_e)
    slot = np.where(
        is_full,
        call_off[e_s, e_c] + tr_off[e_s, e_c, e_tr] + within,
        rest_off[e_s, e_c] + rest_pos,
    )

    src_s = src[order]
    dst_s = dst[order]
    prow_s = _perm_rows(src_s)

    src_rel_pad = np.zeros((NCORES, e_pad), dtype=np.int16)
    dst_loc_pad = np.full((NCORES, e_pad), 0.0, dtype=np.float32)
    # default pad value per slot: 200 for full-block slots, 600 for merged
    is_merged_slot = np.zeros(e_pad, dtype=bool)
    for si in range(NSUP):
        for ci in range(NCHUNKS):
            lo = rest_off[si, ci]
            hi = call_off[si, ci] + 128 * nb_sc[si, ci]
            is_merged_slot[lo:hi] = True
    dst_loc_pad[:, :] = np.where(is_merged_slot[None, :], 600.0, PAD_DST)

    c_s = np.searchsorted(QGOFF, prow_s, side="right") - 1
    src_rel_pad[e_core, slot] = (prow_s - QGOFF[c_s]).astype(np.int16)
    loc_s = dst_s % SHARD
    dstval = np.where(is_full, (loc_s % 128).astype(np.float64), (loc_s % 512))
    dst_loc_pad[e_core, slot] = dstval.astype(np.float32)

    idx16 = np.ascontiguousarray(
        np.tile(src_rel_pad.reshape(NCORES, e_pad // 16, 16).transpose(0, 2, 1), (1, 8, 1))
    )  # [NCORES, 128, e_pad//16]
    dstloc = np.ascontiguousarray(
        dst_loc_pad.reshape(NCORES, nblk, 128).transpose(0, 2, 1)
    )  # [NCORES, 128, nblk]

    sup_cols = []  # supertile -> (start block col, total blocks)
    bbase = 0
    for si in range(NSUP):
        tot = int(nb_sc[si].sum())
        sup_cols.append((bbase, tot))
        bbase += tot
    assert bbase == nblk

    return {
        "idx16": idx16,
        "dstloc": dstloc,
        "nblk": nblk,
        "e_pad": e_pad,
        "nbfull": nbfull,
        "nmerge": nmerge,
        "nb_sc": nb_sc,
        "numidx": numidx,
        "sup_cols": sup_cols,
    }


def _build_pool_layout(batch):
    batch = np.asarray(batch, dtype=np.int64)
    slot_graphs = []  # per core: list of graph ids
    slot_ranges = []  # per core: list of (start, count) local node ranges
    for m in range(NCORES):
        bm = batch[m * SHARD : (m + 1) * SHARD]
        gs, starts, cnts = np.unique(bm, return_index=True, return_counts=True)
        slot_graphs.append(list(gs))
        slot_ranges.append(list(zip(starts.tolist(), cnts.tolist())))
    nslots = max(len(g) for g in slot_graphs)
    nchk = max(
        -(-cnt // 128) for rs in slot_ranges for (_, cnt) in rs
    )  # chunks of 128 per slot

    pool_ids = np.zeros((NCORES, nslots * nchk * 128), dtype=np.int16)
    for m in range(NCORES):
        for j in range(nslots):
            base = j * nchk * 128
            if j < len(slot_ranges[m]):
                start, cnt = slot_ranges[m][j]
                ids = np.full(nchk * 128, start, dtype=np.int16)
                ids[:cnt] = np.arange(start, start + cnt, dtype=np.int16)
            else:
                ids = np.zeros(nchk * 128, dtype=np.int16)
            pool_ids[m, base : base + nchk * 128] = ids

    npool = nslots * nchk * 128
    pool_idx16 = np.ascontiguousarray(
        np.tile(pool_ids.reshape(NCORES, npool // 16, 16).transpose(0, 2, 1), (1, 8, 1))
    )
    return {
        "pool_idx16": pool_idx16,
        "nslots": nslots,
        "nchk": nchk,
        "slot_graphs": slot_graphs,
    }


def _build_nc(lay, pool_lay, sim=False, ablate=(), repeat=1):
    ablate = set(ablate)
    nblk = lay["nblk"]
    e_pad = lay["e_pad"]
    nbfull = lay["nbfull"]
    nmerge = lay["nmerge"]
    nb_sc = lay["nb_sc"]
    numidx = lay["numidx"]
    sup_cols = lay["sup_cols"]
    nslots = pool_lay["nslots"]
    nchk = pool_lay["nchk"]

    nc = bacc.Bacc("TRN2", target_bir_lowering=False, debug=False, num_devices=NCORES,
                   num_swdge_queues=NQUEUES)

    x_in = nc.dram_tensor("x_nm", [N, D], F16, kind="ExternalInput")
    xfm_in = nc.dram_tensor("x_fm", [D, NTILES * 128], F16, kind="ExternalInput")
    idx_in = nc.dram_tensor("idx16", [128, e_pad // 16], I16, kind="ExternalInput")
    dst_in = nc.dram_tensor("dstloc", [128, nblk], F32, kind="ExternalInput")
    w_in = nc.dram_tensor("w", [L * K * 128, 128], F32, kind="ExternalInput")
    sb_in = nc.dram_tensor("scale_bias", [128, 2 * L * K], F32, kind="ExternalInput")
    iota_in = nc.dram_tensor("iota", [128, 512], F16, kind="ExternalInput")
    pidx_in = nc.dram_tensor(
        "pool_idx16", [128, nslots * nchk * 8], I16, kind="ExternalInput"
    )
    pooled_out = nc.dram_tensor("pooled", [128, nslots], F32, kind="ExternalOutput")
    if KM_DEBUG:
        dbg_shard0 = nc.dram_tensor("dbg_shard0", [SHARD, D], F16, kind="ExternalOutput")
        dbg_hnm0 = nc.dram_tensor("dbg_hnm0", [N, D], F16, kind="ExternalOutput")
        dbg_agg0 = nc.dram_tensor("dbg_agg0", [128, 512], F32, kind="ExternalOutput")

    with tile.TileContext(nc) as tc:
        es = ExitStack()
        with es:
            const = es.enter_context(tc.tile_pool(name="const", bufs=1))
            gpool = es.enter_context(tc.tile_pool(name="g", bufs=5))
            ohpool = es.enter_context(tc.tile_pool(name="oh", bufs=8))
            spool = es.enter_context(tc.tile_pool(name="s", bufs=3))
            tnpool = es.enter_context(tc.tile_pool(name="tn", bufs=4))
            stpool = es.enter_context(tc.tile_pool(name="st", bufs=2))
            agg_ps = es.enter_context(tc.tile_pool(name="aggps", bufs=2, space="PSUM"))
            y_ps = es.enter_context(tc.tile_pool(name="yps", bufs=2, space="PSUM"))
            tp_ps = es.enter_context(tc.tile_pool(name="tpps", bufs=2, space="PSUM"))
            dram = es.enter_context(tc.tile_pool(name="dram", bufs=1, space="DRAM"))

            # --- resident SBUF constants
            idx_sb = const.tile([128, e_pad // 16], I16)
            nc.sync.dma_start(out=idx_sb[:], in_=idx_in[:, :])
            dst_sb = const.tile([128, nblk], F32)
            nc.sync.dma_start(out=dst_sb[:], in_=dst_in[:, :])
            iota_sb = const.tile([128, 512], F16)
            nc.sync.dma_start(out=iota_sb[:], in_=iota_in[:, :])
            sb_sb = const.tile([128, 2 * L * K], F32)
            nc.sync.dma_start(out=sb_sb[:], in_=sb_in[:, :])
            pidx_sb = const.tile([128, nslots * nchk * 8], I16)
            nc.sync.dma_start(out=pidx_sb[:], in_=pidx_in[:, :])
            w_sb = []
            for lk in range(L * K):
                w_t = const.tile([128, 128], F32, tag=f"w{lk}", name=f"w{lk}")
                nc.sync.dma_start(out=w_t[:], in_=w_in[lk * 128 : (lk + 1) * 128, :])
                w_sb.append(w_t)
            ident = const.tile([128, 128], F32)
            make_identity(nc, ident[:])

            # zero the gather buffers once: slots beyond a call's num_idxs
            # keep stale SBUF content, which must be finite (0 * onehot-pad)
            for _gz in range(5):
                gz = gpool.tile([128, GMAX, 128], F16, tag="g")
                nc.vector.memset(gz[:], 0.0)

            # --- resident feature-major h slab (f16), seeded with x
            hfm_sb = const.tile([128, NTILES * 128], F16)
            nc.sync.dma_start(out=hfm_sb[:], in_=xfm_in[:, :])

            # --- internal DRAM: per-quarter node tables so a gather only
            # depends on its own quarter's AllGather, not the whole layer
            h_nm = [
                [
                    dram.tile([int(QSIZE[q]) * NCORES, D], F16,
                              tag=f"hnm{i}q{q}", name=f"hnm{i}q{q}")
                    for q in range(4)
                ]
                for i in range(2)
            ]
            shard_nm = [
                [
                    dram.tile([int(QSIZE[q]), D], F16,
                              tag=f"shard{i}q{q}", name=f"shard{i}q{q}")
                    for q in range(4)
                ]
                for i in range(2)
            ]
            h_pool = dram.tile([SHARD, D], F32, tag="hpool")

            for _rep in range(repeat):
                qrr = [0]
                for l in range(L):
                    chunk_views = []
                    for ci in range(NCHUNKS):
                        if l == 0:
                            lo = int(QGOFF[ci])
                            chunk_views.append(
                                x_in[lo : lo + int(QSIZE[ci]) * NCORES, :]
                            )
                        else:
                            chunk_views.append(h_nm[l - 1][ci][:, :])

                    for si in range(NSUP):
                        bbase, btot = sup_cols[si]
                        agg = None
                        if "aggmm" not in ablate:
                            agg = agg_ps.tile([128, 512], F32, space="PSUM", tag="agg")
                        colbase = bbase * 8
                        sup_off = 0
                        for ci in range(NCHUNKS):
                            nbi = int(nb_sc[si, ci])
                            if nbi == 0:
                                continue
                            # block types in consumption order: tr fulls then
                            # merged rest blocks (-1)
                            blk_tr = []
                            for tr in range(4):
                                blk_tr += [tr] * int(nbfull[si, ci, tr])
                            blk_tr += [-1] * int(nmerge[si, ci])
                            nidx_left = int(numidx[si, ci])
                            # split the chunk's blocks into <=GMAX-block
                            # gathers so several stay in flight across the 4
                            # SWDGE queues
                            for gstart in range(0, nbi, GMAX):
                                gcnt = min(GMAX, nbi - gstart)
                                nidx = min(gcnt * 128, nidx_left)
                                nidx_left -= nidx
                                g_t = None
                                if not ("gather" in ablate and "aggmm" in ablate):
                                    g_t = gpool.tile([128, gcnt, 128], F16, tag="g")
                                if "gather" not in ablate and nidx > 0:
                                    nc.gpsimd.dma_gather(
                                        out_ap=g_t[:],
                                        in_ap=chunk_views[ci],
                                        idxs_ap=idx_sb[:, colbase : colbase + gcnt * 8],
                                        num_idxs=nidx,
                                        num_idxs_reg=nidx,
                                        elem_size=D,
                                        single_packet=False,
                                        queue_num=qrr[0] % NQUEUES,
                                    )
                                    qrr[0] += 1
                                colbase += gcnt * 8
                                for off in range(gcnt):
                                    tr = blk_tr[gstart + off]
                                    bcol = bbase + sup_off
                                    wid = 128 if tr >= 0 else 512
                                    if "onehot" in ablate:
                                        oh = iota_sb
                                    else:
                                        # fp16 one-hot via tensor_scalar: the
                                        # per-partition fp32 scalar operand is
                                        # exempt from the 2-byte/packed checks,
                                        # so this runs in the DVE 2x perf mode
                                        # (a broadcast tensor_tensor does not).
                                        oh = ohpool.tile([128, wid], F16,
                                                         tag=f"oh{wid}")
                                        nc.vector.tensor_scalar(
                                            out=oh[:],
                                            in0=iota_sb[:, :wid],
                                            scalar1=dst_sb[:, bcol : bcol + 1],
                                            scalar2=None,
                                            op0=mybir.AluOpType.is_equal,
                                        )
                                    # One accumulation group per PSUM bank:
                                    # start=True clears has_written for the WHOLE
                                    # bank, so only the supertile's first matmul
                                    # may set it. Per-element has_written then
                                    # overwrites on each region's first write and
                                    # accumulates afterwards.
                                    if "aggmm" not in ablate:
                                        cl = tr * 128 if tr >= 0 else 0
                                        nc.tensor.matmul(
                                            out=agg[:, cl : cl + wid],
                                            lhsT=g_t[:, off, :],
                                            rhs=oh[:, :wid],
                                            start=(sup_off == 0),
                                            stop=(sup_off == btot - 1),
                                            skip_group_check=True,
                                        )
                                    sup_off += 1

                        # residual + MLP (feature-major [128, 512], h resident)
                        hfm_t = hfm_sb[:, si * 512 : (si + 1) * 512]
                        u = spool.tile([128, 512], F32, tag="u")
                        if "aggmm" in ablate:
                            nc.vector.tensor_copy(out=u[:], in_=hfm_t)
                        else:
                            nc.vector.tensor_tensor(
                                out=u[:], in0=hfm_t, in1=agg[:], op=mybir.AluOpType.add
                            )
                        if KM_DEBUG and l == 0 and si == 0:
                            agg_sb = spool.tile([128, 512], F32, tag="aggdbg")
                            nc.vector.tensor_copy(out=agg_sb[:], in_=agg[:])
                            nc.sync.dma_start(out=dbg_agg0[:, :], in_=agg_sb[:])
                        cur = u
                        for k in range(K) if "mlp" not in ablate else []:
                            y = y_ps.tile([128, 512], F32, space="PSUM", tag="y")
                            nc.tensor.matmul(
                                out=y[:], lhsT=w_sb[l * K + k][:], rhs=cur[:],
                                start=True, stop=True,
                            )
                            v = spool.tile([128, 512], F32, tag=f"v{k}")
                            col = 2 * (l * K + k)
                            nc.scalar.activation(
                                out=v[:],
                                in_=y[:],
                                func=mybir.ActivationFunctionType.Relu,
                                scale=sb_sb[:, col : col + 1],
                                bias=sb_sb[:, col + 1 : col + 2],
                            )
                            cur = v

                        if l < L - 1:
                            nc.vector.tensor_copy(
                                out=hfm_sb[:, si * 512 : (si + 1) * 512], in_=cur[:]
                            )
                        # node-major writeback via PE transpose
                        for q in range(4):
                            if "transpose" in ablate:
                                continue
                            gt = si * 4 + q
                            row0 = gt * 128
                            if row0 >= SHARD:
                                continue
                            rows = min(128, SHARD - row0)
                            tp = tp_ps.tile([128, 128], F32, space="PSUM", tag="tp")
                            nc.tensor.transpose(
                                out=tp[:], in_=cur[:, q * 128 : (q + 1) * 128],
                                identity=ident[:],
                            )
                            # fp16 rows for the gather tables, fp32 for h_pool
                            tn = tnpool.tile(
                                [128, 128], F16 if l < L - 1 else F32,
                                tag="tn16" if l < L - 1 else "tn32",
                            )
                            nc.vector.tensor_copy(out=tn[:], in_=tp[:])
                            if l < L - 1:
                                wq = int(np.searchsorted(QSTART, row0, side="right") - 1)
                                wrel = row0 - int(QSTART[wq])
                                nc.sync.dma_start(
                                    out=shard_nm[l][wq][wrel : wrel + rows, :],
                                    in_=tn[:rows, :],
                                )
                            else:
                                nc.sync.dma_start(
                                    out=h_pool[row0 : row0 + rows, :], in_=tn[:rows, :]
                                )

                        if l < L - 1 and si in QSUP_LAST and "transpose" not in ablate:
                            qq = QSUP_LAST.index(si)
                            qsz = int(QSIZE[qq])
                            if sim:
                                nc.sync.dma_start(
                                    out=h_nm[l][qq][0:qsz, :],
                                    in_=shard_nm[l][qq][0:qsz, :],
                                )
                            else:
                                nc.gpsimd.collective_compute(
                                    "AllGather",
                                    mybir.AluOpType.bypass,
                                    replica_groups=[list(range(NCORES))],
                                    ins=[shard_nm[l][qq][0:qsz, :].opt()],
                                    outs=[h_nm[l][qq][0 : qsz * NCORES, :].opt()],
                                )

                # --- pooling: per-slot gather + transpose + reduce_max
                pooled_sb = const.tile([128, nslots], F32)
                for j in range(nslots):
                    pg = gpool.tile([128, nchk, 128], F32, tag="pg", bufs=2)
                    nc.gpsimd.dma_gather(
                        out_ap=pg[:],
                        in_ap=h_pool[:],
                        idxs_ap=pidx_sb[:, j * nchk * 8 : (j + 1) * nchk * 8],
                        num_idxs=nchk * 128,
                        num_idxs_reg=nchk * 128,
                        elem_size=D,
                        single_packet=False,
                        queue_num=qrr[0] % NQUEUES,
                    )
                    qrr[0] += 1
                    stg = stpool.tile([128, nchk * 128], F32, tag="stg")
                    for b in range(nchk):
                        tp = tp_ps.tile([128, 128], F32, space="PSUM", tag="tp")
                        nc.tensor.transpose(
                            out=tp[:], in_=pg[:, b, :], identity=ident[:]
                        )
                        nc.vector.tensor_copy(
                            out=stg[:, b * 128 : (b + 1) * 128], in_=tp[:]
                        )
                    nc.vector.reduce_max(
                        out=pooled_sb[:, j : j + 1], in_=stg[:], axis=mybir.AxisListType.X
                    )
                nc.sync.dma_start(out=pooled_out[:, :], in_=pooled_sb[:])

    nc.compile()
    return nc


def kernel(x, edge_index, batch, Ws, bs, gammas, betas, run_means, run_vars, lin_W, lin_b):
    x = np.asarray(x, dtype=np.float32)
    edge_index = np.asarray(edge_index)
    batch = np.asarray(batch)
    Ws = np.asarray(Ws, dtype=np.float32)
    bs = np.asarray(bs, dtype=np.float32)
    gammas = np.asarray(gammas, dtype=np.float32)
    betas = np.asarray(betas, dtype=np.float32)
    run_means = np.asarray(run_means, dtype=np.float32)
    run_vars = np.asarray(run_vars, dtype=np.float32)
    lin_W = np.asarray(lin_W, dtype=np.float32)
    lin_b = np.asarray(lin_b, dtype=np.float32)

    lay = _build_edge_layout(edge_index)
    pool_lay = _build_pool_layout(batch)

    sig = (lay["nblk"], pool_lay["nslots"], pool_lay["nchk"])
    if sig not in _compiled:
        _compiled[sig] = _build_nc(lay, pool_lay)
    nc = _compiled[sig]

    # host-side folded BN params: relu(y*scale + bias')
    scale = gammas / np.sqrt(run_vars + BN_EPS)  # [L, K, D]
    bias = (bs - run_means) * scale + betas  # [L, K, D]
    sb_arr = np.zeros((128, 2 * L * K), dtype=np.float32)
    w_arr = np.zeros((L * K * 128, 128), dtype=np.float32)
    for l in range(L):
        for k in range(K):
            lk = l * K + k
            sb_arr[:, 2 * lk] = scale[l, k]
            sb_arr[:, 2 * lk + 1] = bias[l, k]
            w_arr[lk * 128 : (lk + 1) * 128, :] = Ws[l, k]

    iota = np.tile(np.arange(512, dtype=np.float16)[None, :], (128, 1))
    x_perm = np.empty((N, D), dtype=np.float16)
    x_perm[_perm_rows(np.arange(N))] = x.astype(np.float16)

    in_maps = []
    for m in range(NCORES):
        xfm = np.zeros((D, NTILES * 128), dtype=np.float16)
        xfm[:, :SHARD] = x[m * SHARD : (m + 1) * SHARD].T.astype(np.float16)
        in_maps.append(
            {
                "x_nm": x_perm,
                "x_fm": xfm,
                "idx16": lay["idx16"][m],
                "dstloc": lay["dstloc"][m],
                "w": w_arr,
                "scale_bias": sb_arr,
                "iota": iota,
                "pool_idx16": pool_lay["pool_idx16"][m],
            }
        )

    trace = os.environ.get("KM_TRACE", "0") == "1"
    res = run_bass_kernel_spmd(
        nc, in_maps, core_ids=list(range(NCORES)), trace=trace
    )
    kernel._last_results = res

    pooled_full = np.full((G, D), -np.inf, dtype=np.float32)
    for m in range(NCORES):
        pm = res.results[m]["pooled"]  # [128, nslots]
        for j, g in enumerate(pool_lay["slot_graphs"][m]):
            pooled_full[g] = np.maximum(pooled_full[g], pm[:, j])

    logits = pooled_full @ lin_W + lin_b
    mx = logits.max(axis=-1, keepdims=True)
    z = logits - mx
    out = z - np.log(np.exp(z).sum(axis=-1, keepdims=True))
    return out.astype(np.float32)



# revision 33
# speedup vs baseline: 1.0130x; 1.0130x over previous
"""GIN message-passing kernel for Trainium2, 8-core SPMD.

Strategy (graph/data parallel, edges partitioned by destination):
 - Core m owns destination nodes [12500*m, 12500*(m+1)).
 - Edges sorted by (dst supertile s, src chunk c, dst tile); per (s, c)
   gather call: tr-pure full blocks sized 128*floor(min_core cnt/128)
   (zero padding by construction), then a per-core contiguous "rest"
   run consumed by merged blocks with a 512-wide one-hot spanning the
   whole supertile. num_idxs is trimmed per call to
   128*sum(full) + max_core(rest), so transferred padding is only the
   cross-core rest imbalance (~2.3%); slots beyond it hold stale-but-
   finite SBUF data that the PAD one-hot value zeroes out.
 - Gather chunks coincide with the AllGather quarter regions of the
   permuted node table (max 28672 rows, int16-indexable), and the
   node-major tables are split into one DRAM tile per quarter, so a
   gather depends only on its own quarter's collective.
 - Segment-sum via PSUM-accumulated matmuls
   agg[feat, dst] += G_block.T @ onehot(dst_local) with fp16 one-hots
   built on DVE via tensor_scalar is_equal against an iota row (128
   wide for tr-pure blocks with PAD=200, 512 wide for merged blocks
   with PAD=600). Gather tables are fp16 (rel err ~2e-3).
 - Feature-major h lives in a resident fp16 SBUF slab (seeded with x,
   rewritten in place each layer); residual add + MLP stay fp32 with
   the 128x128 weights stationary; BN (eval) folded into per-partition
   scale/bias of a Relu activation.
 - Node-major h for the next layer's gather written via PE transpose;
   quarter shards exchanged with an 8-core AllGather collective.
 - Pooling: per-graph-slot gathers (pad indices duplicate a real row of
   the same slot, neutral under max), PE transpose, reduce_max. The
   tiny [64,128] @ [128,4] classifier + log_softmax run on host.

Cost-model notes (TimelineSim): all DMA serializes on one device at
max(bytes x mult / 22.5, 7)/16 ns per descriptor with mult=2 below
512B, so the 3 x ~409k x 256B edge gathers (~1.75 ms) are the wall;
engines (DVE one-hots, PE matmuls, Pool SWDGE prep) all hide under it.
"""

import os
import sys

if "/opt/trn_rl_repo" not in sys.path:
    sys.path.append("/opt/trn_rl_repo")

KM_DEBUG = os.environ.get("KM_DEBUG", "0") == "1"

from contextlib import ExitStack

import numpy as np

from concourse import bacc, mybir, tile
from concourse.bass_utils import run_bass_kernel_spmd
from concourse.masks import make_identity

F32 = mybir.dt.float32
F16 = mybir.dt.float16
I16 = mybir.dt.int16

N = 100_000
E = 3_200_000
D = 128
L = 3
K = 2
G = 64
C = 4
BN_EPS = 1e-5

NCORES = 8
SHARD = N // NCORES  # 12500
NTILES = 100  # 128-node dst tiles per core (12800 padded shard)
NSUP = 25  # supertiles of 512 dst nodes
NCHUNKS = 4  # gather chunks == AllGather quarter regions (max 28672 rows)
PAD_DST = 200.0  # local-dst sentinel for padded edges -> all-zero one-hot
NQUEUES = 4
GMAX = 24  # max 128-edge blocks per dma_gather call

# Shard quarters (supertile-aligned) for split AllGathers that overlap the
# tail of each layer's compute. The node table is stored PERMUTED:
# row(g) = QGOFF[q] + QSIZE[q]*m + (loc - QSTART[q]), where m = g//SHARD,
# loc = g%SHARD, q = quarter of loc. Each quarter's AllGather output is the
# rank-concatenation of that quarter across cores.
QSUP_LAST = [6, 12, 18, 24]  # last supertile of each quarter
QSTART = np.array([0, 3584, 6656, 9728])
QSIZE = np.array([3584, 3072, 3072, 2772])
QGOFF = np.array([0, 28672, 53248, 77824])


def _perm_rows(g, pos_of=None):
    g = np.asarray(g, dtype=np.int64)
    m = g // SHARD
    loc = g % SHARD
    p = pos_of[m, loc] if pos_of is not None else loc
    q = np.searchsorted(QSTART, p, side="right") - 1
    return QGOFF[q] + QSIZE[q] * m + (p - QSTART[q])


# supertile ranges of each quarter (positions are quarter-aligned: QSTART
# and QSIZE are multiples of 512 except the tail supertile 24)
QSUP_RANGE = [(0, 7), (7, 13), (13, 19), (19, 25)]


def _balance_positions(edge_index):
    """Per-core, quarter-preserving permutation of dst positions that
    equalizes the per-(supertile, chunk) edge counts across the 8 cores.

    The transferred descriptor count per (s, c) gather call is the MAX over
    cores, so flattening each core's cell profile toward a common target
    removes most of the cross-core imbalance. Permuting only within AllGather
    quarters keeps every node's table quarter (and hence every edge's source
    chunk) unchanged, so there is no circular dependency on the result.
    """
    src = np.asarray(edge_index[0], dtype=np.int64)
    dst = np.asarray(edge_index[1], dtype=np.int64)
    core = dst // SHARD
    loc = dst % SHARD
    prow = _perm_rows(src)  # identity positions: quarters are invariant
    ch = np.searchsorted(QGOFF, prow, side="right") - 1

    pos_of = np.empty((NCORES, SHARD), dtype=np.int64)
    for m in range(NCORES):
        msk = core == m
        V = np.zeros((SHARD, NCHUNKS), dtype=np.int64)
        np.add.at(V, (loc[msk], ch[msk]), 1)
        for q in range(4):
            s0, s1 = QSUP_RANGE[q]
            locs = np.arange(int(QSTART[q]), int(QSTART[q]) + int(QSIZE[q]))
            cap = np.array([min(512, SHARD - 512 * s) for s in range(s0, s1)],
                           dtype=np.int64)
            Vq = V[locs]
            Tq = Vq.sum(axis=0).astype(np.float64)
            target = cap[:, None] / cap.sum() * Tq[None, :]
            order = np.argsort(-Vq.sum(axis=1), kind="stable")
            load = np.zeros((s1 - s0, NCHUNKS))
            room = cap.copy()
            assign = np.empty(len(locs), dtype=np.int64)
            for d in order:
                ratio = ((load + Vq[d][None, :] - target)
                         / np.maximum(target, 1.0)).max(axis=1)
                ratio[room <= 0] = np.inf
                sp = int(np.argmin(ratio))
                assign[d] = sp
                load[sp] += Vq[d]
                room[sp] -= 1
            # hand out positions supertile by supertile in assignment order
            nxt = np.concatenate([[0], np.cumsum(cap)[:-1]]) + 512 * s0
            nxt = nxt.astype(np.int64)
            fill = nxt.copy()
            for i in range(len(locs)):
                sp = assign[i]
                pos_of[m, locs[i]] = fill[sp]
                fill[sp] += 1
    return pos_of

_compiled = {}


def _build_edge_layout(edge_index):
    """Per (supertile s, chunk c) call layout:

      [tr0 full blocks | tr1 | tr2 | tr3 | merged rest blocks]

    Full blocks are tr-pure (128-wide one-hot, dst value = loc%128, PAD 200).
    Each core contributes exactly 128*nbfull_tr = 128*floor(min_core cnt/128)
    edges per tr, so full blocks carry no padding at all. The remaining edges
    of all four trs form a contiguous per-core "rest" run consumed by merged
    blocks with a 512-wide one-hot (dst value = loc%512, PAD 600). The call's
    num_idxs is trimmed to 128*sum(nbfull) + max_core(rest), so the only
    transferred padding is the cross-core rest imbalance; slots beyond a
    core's rest (and beyond num_idxs) read stale SBUF/pad rows that the PAD
    one-hot value zeroes out.
    """
    src = np.asarray(edge_index[0], dtype=np.int64)
    dst = np.asarray(edge_index[1], dtype=np.int64)

    pos_of = _balance_positions(edge_index)
    core = dst // SHARD
    dloc = pos_of[core, dst - core * SHARD]  # balanced dst position
    t = dloc // 128  # local dst tile, 0..97
    s = t // 4  # supertile
    trel = t - s * 4
    prow = _perm_rows(src, pos_of)
    c = np.searchsorted(QGOFF, prow, side="right") - 1  # chunk == quarter

    key = ((core * NSUP + s) * NCHUNKS + c) * 4 + trel
    order = np.argsort(key, kind="stable")
    ks = key[order]

    ngroups = NCORES * NSUP * NCHUNKS * 4
    cnt = np.bincount(ks, minlength=ngroups).reshape(NCORES, NSUP, NCHUNKS, 4)

    # full blocks per (s, c, tr): floor of the min across cores
    nbfull = cnt.min(axis=0) // 128  # [NSUP, NCHUNKS, 4]
    rest = cnt - 128 * nbfull[None]  # [NCORES, NSUP, NCHUNKS, 4]
    rest_sc = rest.sum(axis=3)  # [NCORES, NSUP, NCHUNKS]
    maxrest = rest_sc.max(axis=0)  # [NSUP, NCHUNKS]
    nmerge = -(-maxrest // 128)  # ceil
    # every 128-col region of each supertile must be written by some matmul
    # (full blocks write their tr; merged blocks write all 512 cols)
    for si in range(NSUP):
        assert (nbfull[si].sum(axis=0) >= 1).all() or nmerge[si].sum() >= 1
    nb_sc = nbfull.sum(axis=2) + nmerge  # blocks per (s, c) call
    numidx = 128 * nbfull.sum(axis=2) + maxrest  # transferred descs per call

    nblk = int(nb_sc.sum())
    e_pad = nblk * 128

    # slot offsets: call-major (s, c); within call: tr fulls then rest
    call_off = np.zeros((NSUP, NCHUNKS), dtype=np.int64)
    call_off.reshape(-1)[1:] = np.cumsum((nb_sc * 128).reshape(-1))[:-1]
    tr_off = np.zeros((NSUP, NCHUNKS, 4), dtype=np.int64)
    tr_off[:, :, 1:] = np.cumsum(128 * nbfull, axis=2)[:, :, :-1]
    rest_off = call_off + 128 * nbfull.sum(axis=2)

    # per-edge slot: edges sorted by key; within (core,s,c,tr) group, first
    # 128*nbfull go to that tr's full slots, remainder appended to the
    # core's rest run (tr-ordered since groups are processed in tr order).
    grp_all = np.bincount(ks, minlength=ngroups)
    gstart = np.zeros(ngroups, dtype=np.int64)
    gstart[1:] = np.cumsum(grp_all)[:-1]
    within = np.arange(len(ks), dtype=np.int64) - gstart[ks]

    e_core = ks // (NSUP * NCHUNKS * 4)
    rem = ks % (NSUP * NCHUNKS * 4)
    e_s = rem // (NCHUNKS * 4)
    e_c = (rem // 4) % NCHUNKS
    e_tr = rem % 4

    nfull_e = 128 * nbfull[e_s, e_c, e_tr]
    is_full = within < nfull_e
    # rest position: edges of earlier trs in the same (core,s,c) that
    # overflowed, plus own overflow index
    rest_cum = np.cumsum(rest, axis=3) - rest  # rest before this tr
    rest_pos = rest_cum[e_core, e_s, e_c, e_tr] + (within - nfull_e)
    slot = np.where(
        is_full,
        call_off[e_s, e_c] + tr_off[e_s, e_c, e_tr] + within,
        rest_off[e_s, e_c] + rest_pos,
    )

    src_s = src[order]
    dst_s = dst[order]
    prow_s = _perm_rows(src_s, pos_of)

    src_rel_pad = np.zeros((NCORES, e_pad), dtype=np.int16)
    dst_loc_pad = np.full((NCORES, e_pad), 0.0, dtype=np.float32)
    # default pad value per slot: 200 for full-block slots, 600 for merged
    is_merged_slot = np.zeros(e_pad, dtype=bool)
    for si in range(NSUP):
        for ci in range(NCHUNKS):
            lo = rest_off[si, ci]
            hi = call_off[si, ci] + 128 * nb_sc[si, ci]
            is_merged_slot[lo:hi] = True
    dst_loc_pad[:, :] = np.where(is_merged_slot[None, :], 600.0, PAD_DST)

    c_s = np.searchsorted(QGOFF, prow_s, side="right") - 1
    src_rel_pad[e_core, slot] = (prow_s - QGOFF[c_s]).astype(np.int16)
    loc_s = pos_of[dst_s // SHARD, dst_s % SHARD]
    dstval = np.where(is_full, (loc_s % 128).astype(np.float64), (loc_s % 512))
    dst_loc_pad[e_core, slot] = dstval.astype(np.float32)

    idx16 = np.ascontiguousarray(
        np.tile(src_rel_pad.reshape(NCORES, e_pad // 16, 16).transpose(0, 2, 1), (1, 8, 1))
    )  # [NCORES, 128, e_pad//16]
    dstloc = np.ascontiguousarray(
        dst_loc_pad.reshape(NCORES, nblk, 128).transpose(0, 2, 1)
    )  # [NCORES, 128, nblk]

    sup_cols = []  # supertile -> (start block col, total blocks)
    bbase = 0
    for si in range(NSUP):
        tot = int(nb_sc[si].sum())
        sup_cols.append((bbase, tot))
        bbase += tot
    assert bbase == nblk

    return {
        "idx16": idx16,
        "dstloc": dstloc,
        "nblk": nblk,
        "e_pad": e_pad,
        "nbfull": nbfull,
        "nmerge": nmerge,
        "nb_sc": nb_sc,
        "numidx": numidx,
        "sup_cols": sup_cols,
        "pos_of": pos_of,
    }


def _build_pool_layout(batch, pos_of=None):
    batch = np.asarray(batch, dtype=np.int64)
    slot_graphs = []  # per core: list of graph ids
    slot_ranges = []  # per core: list of (start, count) local node ranges
    for m in range(NCORES):
        bm = batch[m * SHARD : (m + 1) * SHARD]
        gs, starts, cnts = np.unique(bm, return_index=True, return_counts=True)
        slot_graphs.append(list(gs))
        slot_ranges.append(list(zip(starts.tolist(), cnts.tolist())))
    nslots = max(len(g) for g in slot_graphs)
    nchk = max(
        -(-cnt // 128) for rs in slot_ranges for (_, cnt) in rs
    )  # chunks of 128 per slot

    pool_ids = np.zeros((NCORES, nslots * nchk * 128), dtype=np.int16)
    for m in range(NCORES):
        for j in range(nslots):
            base = j * nchk * 128
            if j < len(slot_ranges[m]):
                start, cnt = slot_ranges[m][j]
                slot_pos = np.arange(start, start + cnt, dtype=np.int64)
                if pos_of is not None:
                    slot_pos = pos_of[m, slot_pos]
                ids = np.full(nchk * 128, slot_pos[0], dtype=np.int16)
                ids[:cnt] = slot_pos.astype(np.int16)
            else:
                ids = np.zeros(nchk * 128, dtype=np.int16)
            pool_ids[m, base : base + nchk * 128] = ids

    npool = nslots * nchk * 128
    pool_idx16 = np.ascontiguousarray(
        np.tile(pool_ids.reshape(NCORES, npool // 16, 16).transpose(0, 2, 1), (1, 8, 1))
    )
    return {
        "pool_idx16": pool_idx16,
        "nslots": nslots,
        "nchk": nchk,
        "slot_graphs": slot_graphs,
    }


def _build_nc(lay, pool_lay, sim=False, ablate=(), repeat=1):
    ablate = set(ablate)
    nblk = lay["nblk"]
    e_pad = lay["e_pad"]
    nbfull = lay["nbfull"]
    nmerge = lay["nmerge"]
    nb_sc = lay["nb_sc"]
    numidx = lay["numidx"]
    sup_cols = lay["sup_cols"]
    nslots = pool_lay["nslots"]
    nchk = pool_lay["nchk"]

    nc = bacc.Bacc("TRN2", target_bir_lowering=False, debug=False, num_devices=NCORES,
                   num_swdge_queues=NQUEUES)

    x_in = nc.dram_tensor("x_nm", [N, D], F16, kind="ExternalInput")
    xfm_in = nc.dram_tensor("x_fm", [D, NTILES * 128], F16, kind="ExternalInput")
    idx_in = nc.dram_tensor("idx16", [128, e_pad // 16], I16, kind="ExternalInput")
    dst_in = nc.dram_tensor("dstloc", [128, nblk], F32, kind="ExternalInput")
    w_in = nc.dram_tensor("w", [L * K * 128, 128], F32, kind="ExternalInput")
    sb_in = nc.dram_tensor("scale_bias", [128, 2 * L * K], F32, kind="ExternalInput")
    iota_in = nc.dram_tensor("iota", [128, 512], F16, kind="ExternalInput")
    pidx_in = nc.dram_tensor(
        "pool_idx16", [128, nslots * nchk * 8], I16, kind="ExternalInput"
    )
    pooled_out = nc.dram_tensor("pooled", [128, nslots], F32, kind="ExternalOutput")
    if KM_DEBUG:
        dbg_shard0 = nc.dram_tensor("dbg_shard0", [SHARD, D], F16, kind="ExternalOutput")
        dbg_hnm0 = nc.dram_tensor("dbg_hnm0", [N, D], F16, kind="ExternalOutput")
        dbg_agg0 = nc.dram_tensor("dbg_agg0", [128, 512], F32, kind="ExternalOutput")

    with tile.TileContext(nc) as tc:
        es = ExitStack()
        with es:
            const = es.enter_context(tc.tile_pool(name="const", bufs=1))
            gpool = es.enter_context(tc.tile_pool(name="g", bufs=6))
            ohpool = es.enter_context(tc.tile_pool(name="oh", bufs=8))
            spool = es.enter_context(tc.tile_pool(name="s", bufs=3))
            tnpool = es.enter_context(tc.tile_pool(name="tn", bufs=4))
            stpool = es.enter_context(tc.tile_pool(name="st", bufs=2))
            agg_ps = es.enter_context(tc.tile_pool(name="aggps", bufs=2, space="PSUM"))
            y_ps = es.enter_context(tc.tile_pool(name="yps", bufs=2, space="PSUM"))
            tp_ps = es.enter_context(tc.tile_pool(name="tpps", bufs=2, space="PSUM"))
            dram = es.enter_context(tc.tile_pool(name="dram", bufs=1, space="DRAM"))

            # --- resident SBUF constants
            idx_sb = const.tile([128, e_pad // 16], I16)
            nc.sync.dma_start(out=idx_sb[:], in_=idx_in[:, :])
            dst_sb = const.tile([128, nblk], F32)
            nc.sync.dma_start(out=dst_sb[:], in_=dst_in[:, :])
            iota_sb = const.tile([128, 512], F16)
            nc.sync.dma_start(out=iota_sb[:], in_=iota_in[:, :])
            sb_sb = const.tile([128, 2 * L * K], F32)
            nc.sync.dma_start(out=sb_sb[:], in_=sb_in[:, :])
            pidx_sb = const.tile([128, nslots * nchk * 8], I16)
            nc.sync.dma_start(out=pidx_sb[:], in_=pidx_in[:, :])
            w_sb = []
            for lk in range(L * K):
                w_t = const.tile([128, 128], F32, tag=f"w{lk}", name=f"w{lk}")
                nc.sync.dma_start(out=w_t[:], in_=w_in[lk * 128 : (lk + 1) * 128, :])
                w_sb.append(w_t)
            ident = const.tile([128, 128], F32)
            make_identity(nc, ident[:])

            # zero the gather buffers once: slots beyond a call's num_idxs
            # keep stale SBUF content, which must be finite (0 * onehot-pad)
            for _gz in range(6):
                gz = gpool.tile([128, GMAX, 128], F16, tag="g")
                nc.vector.memset(gz[:], 0.0)

            # --- resident feature-major h slab (f16), seeded with x
            hfm_sb = const.tile([128, NTILES * 128], F16)
            nc.sync.dma_start(out=hfm_sb[:], in_=xfm_in[:, :])

            # --- internal DRAM: per-quarter node tables so a gather only
            # depends on its own quarter's AllGather, not the whole layer
            h_nm = [
                [
                    dram.tile([int(QSIZE[q]) * NCORES, D], F16,
                              tag=f"hnm{i}q{q}", name=f"hnm{i}q{q}")
                    for q in range(4)
                ]
                for i in range(2)
            ]
            shard_nm = [
                [
                    dram.tile([int(QSIZE[q]), D], F16,
                              tag=f"shard{i}q{q}", name=f"shard{i}q{q}")
                    for q in range(4)
                ]
                for i in range(2)
            ]
            h_pool = dram.tile([SHARD, D], F32, tag="hpool")

            for _rep in range(repeat):
                qrr = [0]
                for l in range(L):
                    chunk_views = []
                    for ci in range(NCHUNKS):
                        if l == 0:
                            lo = int(QGOFF[ci])
                            chunk_views.append(
                                x_in[lo : lo + int(QSIZE[ci]) * NCORES, :]
                            )
                        else:
                            chunk_views.append(h_nm[l - 1][ci][:, :])

                    for si in range(NSUP):
                        bbase, btot = sup_cols[si]
                        agg = None
                        if "aggmm" not in ablate:
                            agg = agg_ps.tile([128, 512], F32, space="PSUM", tag="agg")
                        colbase = bbase * 8
                        sup_off = 0
                        for ci in range(NCHUNKS):
                            nbi = int(nb_sc[si, ci])
                            if nbi == 0:
                                continue
                            # block types in consumption order: tr fulls then
                            # merged rest blocks (-1)
                            blk_tr = []
                            for tr in range(4):
                                blk_tr += [tr] * int(nbfull[si, ci, tr])
                            blk_tr += [-1] * int(nmerge[si, ci])
                            nidx_left = int(numidx[si, ci])
                            # split the chunk's blocks into <=GMAX-block
                            # gathers so several stay in flight across the 4
                            # SWDGE queues
                            for gstart in range(0, nbi, GMAX):
                                gcnt = min(GMAX, nbi - gstart)
                                nidx = min(gcnt * 128, nidx_left)
                                nidx_left -= nidx
                                g_t = None
                                if not ("gather" in ablate and "aggmm" in ablate):
                                    g_t = gpool.tile([128, gcnt, 128], F16, tag="g")
                                if "gather" not in ablate and nidx > 0:
                                    nc.gpsimd.dma_gather(
                                        out_ap=g_t[:],
                                        in_ap=chunk_views[ci],
                                        idxs_ap=idx_sb[:, colbase : colbase + gcnt * 8],
                                        num_idxs=nidx,
                                        num_idxs_reg=nidx,
                                        elem_size=D,
                                        single_packet=False,
                                        queue_num=qrr[0] % NQUEUES,
                                    )
                                    qrr[0] += 1
                                colbase += gcnt * 8
                                for off in range(gcnt):
                                    tr = blk_tr[gstart + off]
                                    bcol = bbase + sup_off
                                    wid = 128 if tr >= 0 else 512
                                    if "onehot" in ablate:
                                        oh = iota_sb
                                    else:
                                        # fp16 one-hot via tensor_scalar: the
                                        # per-partition fp32 scalar operand is
                                        # exempt from the 2-byte/packed checks,
                                        # so this runs in the DVE 2x perf mode
                                        # (a broadcast tensor_tensor does not).
                                        oh = ohpool.tile([128, wid], F16,
                                                         tag=f"oh{wid}")
                                        nc.vector.tensor_scalar(
                                            out=oh[:],
                                            in0=iota_sb[:, :wid],
                                            scalar1=dst_sb[:, bcol : bcol + 1],
                                            scalar2=None,
                                            op0=mybir.AluOpType.is_equal,
                                        )
                                    # One accumulation group per PSUM bank:
                                    # start=True clears has_written for the WHOLE
                                    # bank, so only the supertile's first matmul
                                    # may set it. Per-element has_written then
                                    # overwrites on each region's first write and
                                    # accumulates afterwards.
                                    if "aggmm" not in ablate:
                                        cl = tr * 128 if tr >= 0 else 0
                                        nc.tensor.matmul(
                                            out=agg[:, cl : cl + wid],
                                            lhsT=g_t[:, off, :],
                                            rhs=oh[:, :wid],
                                            start=(sup_off == 0),
                                            stop=(sup_off == btot - 1),
                                            skip_group_check=True,
                                        )
                                    sup_off += 1

                        # residual + MLP (feature-major [128, 512], h resident)
                        hfm_t = hfm_sb[:, si * 512 : (si + 1) * 512]
                        u = spool.tile([128, 512], F32, tag="u")
                        if "aggmm" in ablate:
                            nc.vector.tensor_copy(out=u[:], in_=hfm_t)
                        else:
                            nc.vector.tensor_tensor(
                                out=u[:], in0=hfm_t, in1=agg[:], op=mybir.AluOpType.add
                            )
                        if KM_DEBUG and l == 0 and si == 0:
                            agg_sb = spool.tile([128, 512], F32, tag="aggdbg")
                            nc.vector.tensor_copy(out=agg_sb[:], in_=agg[:])
                            nc.sync.dma_start(out=dbg_agg0[:, :], in_=agg_sb[:])
                        cur = u
                        for k in range(K) if "mlp" not in ablate else []:
                            y = y_ps.tile([128, 512], F32, space="PSUM", tag="y")
                            nc.tensor.matmul(
                                out=y[:], lhsT=w_sb[l * K + k][:], rhs=cur[:],
                                start=True, stop=True,
                            )
                            v = spool.tile([128, 512], F32, tag=f"v{k}")
                            col = 2 * (l * K + k)
                            nc.scalar.activation(
                                out=v[:],
                                in_=y[:],
                                func=mybir.ActivationFunctionType.Relu,
                                scale=sb_sb[:, col : col + 1],
                                bias=sb_sb[:, col + 1 : col + 2],
                            )
                            cur = v

                        if l < L - 1:
                            nc.vector.tensor_copy(
                                out=hfm_sb[:, si * 512 : (si + 1) * 512], in_=cur[:]
                            )
                        # node-major writeback via PE transpose
                        for q in range(4):
                            if "transpose" in ablate:
                                continue
                            gt = si * 4 + q
                            row0 = gt * 128
                            if row0 >= SHARD:
                                continue
                            rows = min(128, SHARD - row0)
                            tp = tp_ps.tile([128, 128], F32, space="PSUM", tag="tp")
                            nc.tensor.transpose(
                                out=tp[:], in_=cur[:, q * 128 : (q + 1) * 128],
                                identity=ident[:],
                            )
                            # fp16 rows for the gather tables, fp32 for h_pool
                            tn = tnpool.tile(
                                [128, 128], F16 if l < L - 1 else F32,
                                tag="tn16" if l < L - 1 else "tn32",
                            )
                            nc.vector.tensor_copy(out=tn[:], in_=tp[:])
                            if l < L - 1:
                                wq = int(np.searchsorted(QSTART, row0, side="right") - 1)
                                wrel = row0 - int(QSTART[wq])
                                nc.sync.dma_start(
                                    out=shard_nm[l][wq][wrel : wrel + rows, :],
                                    in_=tn[:rows, :],
                                )
                            else:
                                nc.sync.dma_start(
                                    out=h_pool[row0 : row0 + rows, :], in_=tn[:rows, :]
                                )

                        if l < L - 1 and si in QSUP_LAST and "transpose" not in ablate:
                            qq = QSUP_LAST.index(si)
                            qsz = int(QSIZE[qq])
                            if sim:
                                nc.sync.dma_start(
                                    out=h_nm[l][qq][0:qsz, :],
                                    in_=shard_nm[l][qq][0:qsz, :],
                                )
                            else:
                                nc.gpsimd.collective_compute(
                                    "AllGather",
                                    mybir.AluOpType.bypass,
                                    replica_groups=[list(range(NCORES))],
                                    ins=[shard_nm[l][qq][0:qsz, :].opt()],
                                    outs=[h_nm[l][qq][0 : qsz * NCORES, :].opt()],
                                )

                # --- pooling: per-slot gather + transpose + reduce_max
                pooled_sb = const.tile([128, nslots], F32)
                for j in range(nslots):
                    pg = gpool.tile([128, nchk, 128], F32, tag="pg", bufs=2)
                    nc.gpsimd.dma_gather(
                        out_ap=pg[:],
                        in_ap=h_pool[:],
                        idxs_ap=pidx_sb[:, j * nchk * 8 : (j + 1) * nchk * 8],
                        num_idxs=nchk * 128,
                        num_idxs_reg=nchk * 128,
                        elem_size=D,
                        single_packet=False,
                        queue_num=qrr[0] % NQUEUES,
                    )
                    qrr[0] += 1
                    stg = stpool.tile([128, nchk * 128], F32, tag="stg")
                    for b in range(nchk):
                        tp = tp_ps.tile([128, 128], F32, space="PSUM", tag="tp")
                        nc.tensor.transpose(
                            out=tp[:], in_=pg[:, b, :], identity=ident[:]
                        )
                        nc.vector.tensor_copy(
                            out=stg[:, b * 128 : (b + 1) * 128], in_=tp[:]
                        )
                    nc.vector.reduce_max(
                        out=pooled_sb[:, j : j + 1], in_=stg[:], axis=mybir.AxisListType.X
                    )
                nc.sync.dma_start(out=pooled_out[:, :], in_=pooled_sb[:])

    nc.compile()
    return nc


def kernel(x, edge_index, batch, Ws, bs, gammas, betas, run_means, run_vars, lin_W, lin_b):
    x = np.asarray(x, dtype=np.float32)
    edge_index = np.asarray(edge_index)
    batch = np.asarray(batch)
    Ws = np.asarray(Ws, dtype=np.float32)
    bs = np.asarray(bs, dtype=np.float32)
    gammas = np.asarray(gammas, dtype=np.float32)
    betas = np.asarray(betas, dtype=np.float32)
    run_means = np.asarray(run_means, dtype=np.float32)
    run_vars = np.asarray(run_vars, dtype=np.float32)
    lin_W = np.asarray(lin_W, dtype=np.float32)
    lin_b = np.asarray(lin_b, dtype=np.float32)

    lay = _build_edge_layout(edge_index)
    pool_lay = _build_pool_layout(batch, lay["pos_of"])

    sig = (lay["nblk"], pool_lay["nslots"], pool_lay["nchk"])
    if sig not in _compiled:
        _compiled[sig] = _build_nc(lay, pool_lay)
    nc = _compiled[sig]

    # host-side folded BN params: relu(y*scale + bias')
    scale = gammas / np.sqrt(run_vars + BN_EPS)  # [L, K, D]
    bias = (bs - run_means) * scale + betas  # [L, K, D]
    sb_arr = np.zeros((128, 2 * L * K), dtype=np.float32)
    w_arr = np.zeros((L * K * 128, 128), dtype=np.float32)
    for l in range(L):
        for k in range(K):
            lk = l * K + k
            sb_arr[:, 2 * lk] = scale[l, k]
            sb_arr[:, 2 * lk + 1] = bias[l, k]
            w_arr[lk * 128 : (lk + 1) * 128, :] = Ws[l, k]

    iota = np.tile(np.arange(512, dtype=np.float16)[None, :], (128, 1))
    pos_of = lay["pos_of"]
    x_perm = np.empty((N, D), dtype=np.float16)
    x_perm[_perm_rows(np.arange(N), pos_of)] = x.astype(np.float16)

    in_maps = []
    for m in range(NCORES):
        xfm = np.zeros((D, NTILES * 128), dtype=np.float16)
        xfm[:, pos_of[m]] = x[m * SHARD : (m + 1) * SHARD].T.astype(np.float16)
        in_maps.append(
            {
                "x_nm": x_perm,
                "x_fm": xfm,
                "idx16": lay["idx16"][m],
                "dstloc": lay["dstloc"][m],
                "w": w_arr,
                "scale_bias": sb_arr,
                "iota": iota,
                "pool_idx16": pool_lay["pool_idx16"][m],
            }
        )

    trace = os.environ.get("KM_TRACE", "0") == "1"
    res = run_bass_kernel_spmd(
        nc, in_maps, core_ids=list(range(NCORES)), trace=trace
    )
    kernel._last_results = res

    pooled_full = np.full((G, D), -np.inf, dtype=np.float32)
    for m in range(NCORES):
        pm = res.results[m]["pooled"]  # [128, nslots]
        for j, g in enumerate(pool_lay["slot_graphs"][m]):
            pooled_full[g] = np.maximum(pooled_full[g], pm[:, j])

    logits = pooled_full @ lin_W + lin_b
    mx = logits.max(axis=-1, keepdims=True)
    z = logits - mx
    out = z - np.log(np.exp(z).sum(axis=-1, keepdims=True))
    return out.astype(np.float32)

